# revision 30
# baseline (speedup 1.0000x reference)
"""Trainium2 Bass kernel for the LRU LM (nn_LruLM), v4.

Sharding: each core takes chunk k of BOTH batches (8 chunks of 256 per batch),
processed MERGED as one [*, 512] tile (b0 cols 0:256 | b1 cols 256:512), so all
weight matmuls run with N=512 moving operands (LDWEIGHTS fully amortized).
The LRU scan runs on merged tiles with a zero multiplier at the batch boundary
column. One boundary-state AllGather per layer (both batches), covered by the
in_proj o-region matmuls. LayerNorm uses broadcast-native stats (all-ones
128x128 stationary -> per-token sums broadcast to all partitions) with
reciprocal_approx_fast; all LN gains/biases are folded into adjacent weights
host-side. Logits are vocab-sharded with pw-stationary / activation-moving
matmuls, LDWEIGHTS amortized over token blocks; the final activation AllGather
is covered by the core's own-token-block logits pass.
"""

import contextlib

import numpy as np
import ml_dtypes

import concourse.bacc as bacc
import concourse.mybir as mybir
import concourse.tile as tile
from concourse.bass_utils import run_bass_kernel_spmd

AF = mybir.ActivationFunctionType
OP = mybir.AluOpType
F32 = mybir.dt.float32
F32R = mybir.dt.float32r
BF16 = mybir.dt.bfloat16

V, D, L, B, S = 50257, 768, 6, 2, 2048
TC = 256                     # tokens per chunk per batch
T = 2 * TC                   # tokens per core (b0 cols | b1 cols)
NC = 8
NCH = 8                      # chunks per batch
CT = D // 128                # 6 channel tiles
VSHP = 6400                  # padded vocab shard width (50 * 128)
VSH = 6284                   # true vocab shard width
VTN = VSHP // 128            # 50 vocab tiles
VGRP = 25                    # vocab tiles per pw group (2 groups)
EPS = 1e-5
ALL8 = [list(range(NC))]
# in_proj column order: (vr_i, vi_i) pairs first so each pair's rotation +
# scan starts as soon as its two psums land; o tiles afterwards.
PERM = [0, 6, 1, 7, 2, 8, 3, 9, 4, 10, 5, 11] + list(range(12, 24))


def _build(nc):
    d = {}
    d["x0t"] = nc.dram_tensor("x0t", [D, T], F32R, kind="ExternalInput")
    d["postc"] = nc.dram_tensor("postc", [L, D, TC], BF16, kind="ExternalInput")
    d["posts"] = nc.dram_tensor("posts", [L, D, TC], BF16, kind="ExternalInput")
    d["npwt"] = nc.dram_tensor("npwt", [L, D, TC], BF16, kind="ExternalInput")
    d["bmask"] = nc.dram_tensor("bmask", [128, T], F32, kind="ExternalInput")
    d["cw"] = nc.dram_tensor("cw", [L, CT, 128, NCH], F32, kind="ExternalInput")
    d["nuv"] = nc.dram_tensor("nuv", [128, CT * L], F32, kind="ExternalInput")
    for nm in ["outbv", "b2v"]:
        d[nm] = nc.dram_tensor(nm, [128, CT * L], F32, kind="ExternalInput")
    for nm in ["inbv", "b1v"]:
        d[nm] = nc.dram_tensor(nm, [128, 24 * L], F32, kind="ExternalInput")
    d["w_in"] = nc.dram_tensor("w_in", [L, CT, 128, 24 * 128], BF16, kind="ExternalInput")
    d["w_out"] = nc.dram_tensor("w_out", [L, 2 * CT, 128, CT * 128], BF16, kind="ExternalInput")
    d["w_1"] = nc.dram_tensor("w_1", [L, CT, 128, 24 * 128], BF16, kind="ExternalInput")
    d["w_2"] = nc.dram_tensor("w_2", [L, 24, 128, CT * 128], BF16, kind="ExternalInput")
    d["pwt"] = nc.dram_tensor("pwt", [CT, 128, VSHP], BF16, kind="ExternalInput")
    outp = nc.dram_tensor("outp", [VSHP, NC * T], BF16, kind="ExternalOutput")

    cc_in = [nc.dram_tensor(f"ccin{l}", [128, 4 * CT], F32) for l in range(L)]
    cc_out = [nc.dram_tensor(f"ccout{l}", [NCH * 128, 4 * CT], F32)
              for l in range(L)]
    xf_in = nc.dram_tensor("xfin", [D, T], BF16)
    xf_all = nc.dram_tensor("xfall", [NC * D, T], BF16, addr_space="Shared")

    with tile.TileContext(nc) as tc:
        est = contextlib.ExitStack()
        with est:
            vec = est.enter_context(tc.tile_pool(name="vec", bufs=1))
            tmp = est.enter_context(tc.tile_pool(name="tmp", bufs=2))
            rsp = est.enter_context(tc.tile_pool(name="rsp", bufs=2))
            ps_st = est.enter_context(tc.tile_pool(name="psst", bufs=1, space="PSUM"))
            ps_mm = est.enter_context(tc.tile_pool(name="psmm", bufs=6, space="PSUM"))

            ones128f = vec.tile([128, 128], F32, tag="ones128f")
            nc.vector.memset(ones128f[:], 1.0)
            ones128r = vec.tile([128, 128], F32R, tag="ones128r")
            nc.vector.tensor_copy(ones128r[:], ones128f[:])
            epst = vec.tile([128, 1], F32, tag="epst")
            nc.vector.memset(epst[:], EPS)
            ones128b = vec.tile([128, 128], BF16, tag="ones128b")
            nc.vector.memset(ones128b[:], 1.0)
            bmask = vec.tile([128, T], F32, tag="bmask")
            nc.sync.dma_start(bmask[:], d["bmask"][:])
            cwt = vec.tile([128, L * CT * NCH], F32, tag="cwt")
            nc.sync.dma_start(
                cwt[:].rearrange("p (l c j) -> p l c j", l=L, c=CT),
                d["cw"][:].rearrange("l c p j -> p l c j"),
            )

            vt = {}
            for nm in ["nuv", "outbv", "b2v", "inbv", "b1v"]:
                vt[nm] = vec.tile(list(d[nm].shape), F32, tag=nm, name=nm)
                nc.sync.dma_start(vt[nm][:], d[nm][:])

            def do_ln(xaps, out_pool, out_tag, order=None):
                """Broadcast-native LN over channels (partitions across the
                len(xaps) [128,T] APs). Stats via all-ones [128,128]
                stationary matmuls on bf16 copies -> per-token sums broadcast
                to every partition; normalize in bf16 2x DVE mode. `order`
                visits tiles in production order so chains start early."""
                n = len(xaps)
                nch = float(n * 128)
                ord_ = list(range(n)) if order is None else order
                is_bf = xaps[0].dtype == BF16
                if is_bf:
                    xb = xaps
                else:
                    xb = [None] * n
                    for i in ord_:
                        c = tmp.tile([128, T], BF16, tag="lnxb", bufs=6)
                        nc.scalar.activation(c[:], xaps[i], AF.Copy)
                        xb[i] = c[:]
                bcS = ps_st.tile([128, T], F32, tag="bcS")
                bcQ = ps_st.tile([128, T], F32, tag="bcQ")
                for idx, i in enumerate(ord_):
                    nc.tensor.matmul(bcS[:], ones128b[:], xb[i],
                                     start=(idx == 0), stop=(idx == n - 1))
                for idx, i in enumerate(ord_):
                    sq = tmp.tile([128, T], BF16, tag="sqt", bufs=2)
                    nc.scalar.activation(sq[:], xb[i], AF.Square)
                    nc.tensor.matmul(bcQ[:], ones128b[:], sq[:],
                                     start=(idx == 0), stop=(idx == n - 1))
                m = rsp.tile([128, T], F32, tag="lnm")
                nc.vector.tensor_scalar_mul(m[:], bcS[:], 1.0 / nch)
                s2 = tmp.tile([128, T], F32, tag="lns2", bufs=1)
                nc.scalar.activation(s2[:], m[:], AF.Square)
                var = tmp.tile([128, T], F32, tag="lnvar", bufs=1)
                nc.vector.scalar_tensor_tensor(var[:], bcQ[:], 1.0 / nch,
                                               s2[:], OP.mult, OP.subtract)
                sd = rsp.tile([128, T], F32, tag="lnsd", bufs=1)
                nc.scalar.activation(sd[:], var[:], AF.Sqrt, bias=epst[:])
                rstd = rsp.tile([128, T], F32, tag="lnrstd")
                nc.vector.reciprocal_approx_fast(rstd[:], sd[:])
                outs = [None] * n
                if is_bf:
                    # bf16 inputs: stay in bf16 2x mode end-to-end
                    mb = rsp.tile([128, T], BF16, tag="lnmb", bufs=1)
                    nc.scalar.activation(mb[:], m[:], AF.Copy)
                    rstdb = rsp.tile([128, T], BF16, tag="lnrstdb", bufs=1)
                    nc.scalar.activation(rstdb[:], rstd[:], AF.Copy)
                    for i in ord_:
                        t1 = tmp.tile([128, T], BF16, tag="lnt1", bufs=2)
                        nc.vector.tensor_tensor(t1[:], xb[i], mb[:], OP.subtract)
                        o = out_pool.tile([128, T], BF16, tag=out_tag)
                        nc.vector.tensor_tensor(o[:], t1[:], rstdb[:], OP.mult)
                        outs[i] = o
                else:
                    # f32r residual: full-precision subtract/scale, bf16 out
                    for i in ord_:
                        t1 = tmp.tile([128, T], F32, tag="lnt1f", bufs=2)
                        nc.vector.tensor_tensor(t1[:], xaps[i], m[:],
                                                OP.subtract)
                        o = out_pool.tile([128, T], BF16, tag=out_tag)
                        nc.vector.tensor_tensor(o[:], t1[:], rstd[:], OP.mult)
                        outs[i] = o
                return outs

            def proj(n_k, n_mt, get_w, rhs_list, grp, group_cb, k_order=None):
                """psum[mi] = sum_k get_w(k, m0+mi).T @ rhs_list[k].
                k_order lets accumulation visit k-tiles in the order their
                rhs tiles are produced, so chains start early."""
                ks = list(range(n_k)) if k_order is None else k_order
                for m0 in range(0, n_mt, grp):
                    g = min(grp, n_mt - m0)
                    psl = [ps_mm.tile([128, T], F32, tag="mmps", name="mmps")
                           for _ in range(g)]
                    for idx, ki in enumerate(ks):
                        for mi in range(g):
                            nc.tensor.matmul(psl[mi][:], get_w(ki, m0 + mi),
                                             rhs_list[ki][:],
                                             start=(idx == 0),
                                             stop=(idx == n_k - 1))
                    group_cb(m0, psl)

            tst = contextlib.ExitStack()
            with tst:
                xres = tst.enter_context(tc.tile_pool(name="xres", bufs=12))
                rhs = tst.enter_context(tc.tile_pool(name="rhs", bufs=18))
                tabs = tst.enter_context(tc.tile_pool(name="tabs", bufs=1))
                nubp = tst.enter_context(tc.tile_pool(name="nubp", bufs=1))
                abv = tst.enter_context(tc.tile_pool(name="abv", bufs=2))
                ABp = tst.enter_context(tc.tile_pool(name="ABp", bufs=6))
                sop = tst.enter_context(tc.tile_pool(name="sop", bufs=12))
                ypp = tst.enter_context(tc.tile_pool(name="ypp", bufs=12))
                cpp = tst.enter_context(tc.tile_pool(name="cpp", bufs=3))
                scl = tst.enter_context(tc.tile_pool(name="scl", bufs=2))
                w3k = tst.enter_context(tc.tile_pool(name="w3k", bufs=6))
                w15 = tst.enter_context(tc.tile_pool(name="w15", bufs=13))
                mpp = tst.enter_context(tc.tile_pool(name="mpp", bufs=6))

                x = []
                for i in range(CT):
                    xt = xres.tile([128, T], F32R, tag="x")
                    nc.sync.dma_start(xt[:], d["x0t"][i * 128:(i + 1) * 128, :])
                    x.append(xt)

                def load_w15(dram, l, kts, c0, c1):
                    tiles = []
                    for kt in kts:
                        wt = w15.tile([128, c1 - c0], BF16, tag="w15", name="w15t")
                        nc.sync.dma_start(wt[:], dram[l, kt, :, c0:c1])
                        tiles.append(wt)
                    return tiles

                def load_whalf(dram, l, hh):
                    tiles = []
                    for kt in range(CT):
                        wt = w3k.tile([128, 1536], BF16, tag="w3k", name="w3k")
                        nc.sync.dma_start(
                            wt[:], dram[l, kt, :, hh * 1536:(hh + 1) * 1536])
                        tiles.append(wt)
                    return tiles

                for l in range(L):
                    co = l * CT
                    pct = tabs.tile([128, CT * TC], BF16, tag="pct", name="pct")
                    nc.sync.dma_start(
                        pct[:].rearrange("p (c t) -> p c t", c=CT),
                        d["postc"][l].rearrange("(c p) t -> p c t", p=128),
                    )
                    psnt = tabs.tile([128, CT * TC], BF16, tag="psnt", name="psnt")
                    nc.sync.dma_start(
                        psnt[:].rearrange("p (c t) -> p c t", c=CT),
                        d["posts"][l].rearrange("(c p) t -> p c t", p=128),
                    )
                    npwt = tabs.tile([128, CT * TC], BF16, tag="npwt", name="npwt")
                    nc.sync.dma_start(
                        npwt[:].rearrange("p (c t) -> p c t", c=CT),
                        d["npwt"][l].rearrange("(c p) t -> p c t", p=128),
                    )

                    AB = {}
                    so = {}
                    inis = {}
                    xn = [None] * CT

                    def pc_i(i):
                        # [128, 2, TC] broadcast AP: same table for both halves
                        return pct[:, i * TC:(i + 1) * TC][:, None, :].broadcast_to(
                            (128, 2, TC))

                    def psn_i(i):
                        return psnt[:, i * TC:(i + 1) * TC][:, None, :].broadcast_to(
                            (128, 2, TC))

                    def npw_i(i):
                        return npwt[:, i * TC:(i + 1) * TC]

                    endAB = scl.tile([128, 4 * CT], F32, tag="endAB", bufs=2,
                                     name="endAB")

                    def rot_scan(i, pvr, pvi, l=l, AB=AB):
                        nub = nubp.tile([128, T], F32, tag="nub", name="nub")
                        nc.vector.tensor_scalar_mul(
                            nub[:], bmask[:], vt["nuv"][:, co + i:co + i + 1])
                        bvr = vt["inbv"][:, l * 24 + i:l * 24 + i + 1]
                        bvi = vt["inbv"][:, l * 24 + CT + i:l * 24 + CT + i + 1]
                        t1 = tmp.tile([128, T], F32, tag="rt", bufs=4, name="rt1")
                        nc.vector.scalar_tensor_tensor(t1[:], pvr[:], bvr,
                                                       pc_i(i), OP.add, OP.mult)
                        t2 = tmp.tile([128, T], F32, tag="rt", bufs=4, name="rt2")
                        nc.vector.scalar_tensor_tensor(t2[:], pvi[:], bvi,
                                                       psn_i(i), OP.add, OP.mult)
                        av = abv.tile([128, T], F32, tag="av", bufs=2)
                        nc.vector.tensor_tensor(av[:], t1[:], t2[:], OP.add)
                        t3 = tmp.tile([128, T], F32, tag="rt", bufs=4, name="rt3")
                        nc.vector.scalar_tensor_tensor(t3[:], pvr[:], bvr,
                                                       psn_i(i), OP.add, OP.mult)
                        t4 = tmp.tile([128, T], F32, tag="rt", bufs=4, name="rt4")
                        nc.vector.scalar_tensor_tensor(t4[:], pvi[:], bvi,
                                                       pc_i(i), OP.add, OP.mult)
                        bv = abv.tile([128, T], F32, tag="bv", bufs=2)
                        nc.gpsimd.tensor_tensor(bv[:], t3[:], t4[:], OP.subtract)
                        Av = ABp.tile([128, T], BF16, tag="Av")
                        nc.vector.tensor_tensor_scan(Av[:], nub[:], av[:],
                                                     0.0, OP.mult, OP.add)
                        Bv = ABp.tile([128, T], BF16, tag="Bv")
                        nc.vector.tensor_tensor_scan(Bv[:], nub[:], bv[:],
                                                     0.0, OP.mult, OP.add)
                        AB[i] = (Av, Bv)
                        # end states: [A_b0 | B_b0 | A_b1 | B_b1] x CT cols
                        eb4 = endAB[:].rearrange("p (b a c) -> p b a c", b=2, a=2)
                        nc.vector.tensor_copy(
                            eb4[:, :, 0:1, i:i + 1],
                            Av[:].rearrange("p (h t) -> p h t", t=TC)
                            [:, :, TC - 1:TC])
                        nc.vector.tensor_copy(
                            eb4[:, :, 1:2, i:i + 1],
                            Bv[:].rearrange("p (h t) -> p h t", t=TC)
                            [:, :, TC - 1:TC])

                    # ---- ln1 + in_proj ----
                    ln1r = do_ln([x[i][:] for i in range(CT)], rhs, "lnout")
                    w_v = load_whalf(d["w_in"], l, 0)

                    def v_cb(m0, psl):
                        rot_scan(m0 // 2, psl[0], psl[1])

                    proj(CT, 12,
                         lambda ki, mpos: w_v[ki][:, mpos * 128:(mpos + 1) * 128],
                         ln1r, 2, v_cb)
                    # boundary exchange (issued before the o-region mms)
                    nc.sync.dma_start(cc_in[l][:], endAB[:])
                    nc.gpsimd.collective_compute(
                        "AllGather", OP.bypass, replica_groups=ALL8,
                        ins=[cc_in[l][:]], outs=[cc_out[l][:]],
                    )
                    w_o = load_whalf(d["w_in"], l, 1)

                    def o_cb(m0, psl, l=l):
                        for mi, ps in enumerate(psl):
                            mt = PERM[12 + m0 + mi]
                            s = sop.tile([128, T], BF16, tag="so")
                            nc.scalar.activation(
                                s[:], ps[:], AF.Silu,
                                bias=vt["inbv"][:, l * 24 + mt:l * 24 + mt + 1])
                            so[mt - 2 * CT] = s

                    proj(CT, 12,
                         lambda ki, mpos: w_o[ki][:, mpos * 128:(mpos + 1) * 128],
                         ln1r, 3, o_cb)

                    # ---- gather + correction scalars ----
                    gat = scl.tile([128, 4 * CT * NCH], F32, tag="gat",
                                   bufs=2, name="gat")
                    nc.sync.dma_start(
                        gat[:].rearrange("p (c j) -> p c j", j=NCH),
                        cc_out[l][:].rearrange("(j p) c -> p c j", p=128),
                    )
                    for i in range(CT):
                        for b in range(2):
                            for ab in range(2):
                                cwsl = cwt[:, (l * CT + i) * NCH:
                                           (l * CT + i + 1) * NCH]
                                junk = scl.tile([128, NCH], F32, tag="inij",
                                                bufs=2, name="inij")
                                ini = scl.tile([128, 1], F32, tag="ini", bufs=26,
                                               name="ini")
                                c0 = ((b * 2 + ab) * CT + i) * NCH
                                nc.vector.scalar_tensor_tensor(
                                    junk[:], gat[:, c0:c0 + NCH], 1.0, cwsl,
                                    OP.mult, OP.mult, accum_out=ini[:])
                                inis[(i, ab, b)] = ini

                    # ---- post: corrections, rotate back, y ----
                    ys = [None] * (2 * CT)
                    for i in range(CT):
                        Av, Bv = AB[i]
                        c0t = cpp.tile([128, T], BF16, tag="Sc", name="c0")
                        c1t = cpp.tile([128, T], BF16, tag="Sc", name="c1")
                        for b in range(2):
                            hs = slice(b * TC, (b + 1) * TC)
                            nc.vector.scalar_tensor_tensor(
                                c0t[:, hs], npw_i(i), inis[(i, 0, b)][:],
                                Av[:, hs], OP.mult, OP.add)
                            nc.vector.scalar_tensor_tensor(
                                c1t[:, hs], npw_i(i), inis[(i, 1, b)][:],
                                Bv[:, hs], OP.mult, OP.add)
                        t1 = tmp.tile([128, T], BF16, tag="rt", bufs=4, name="pt1")
                        nc.vector.tensor_tensor(t1[:], pc_i(i), c0t[:], OP.mult)
                        t2 = tmp.tile([128, T], BF16, tag="rt", bufs=4, name="pt2")
                        nc.vector.tensor_tensor(t2[:], psn_i(i), c1t[:], OP.mult)
                        hr = abv.tile([128, T], BF16, tag="av", bufs=2, name="hr")
                        nc.vector.tensor_tensor(hr[:], t1[:], t2[:], OP.add)
                        yv = ypp.tile([128, T], BF16, tag="ypart")
                        nc.vector.tensor_tensor(yv[:], hr[:], so[i][:], OP.mult)
                        ys[i] = yv
                        t3 = tmp.tile([128, T], BF16, tag="rt", bufs=4, name="pt3")
                        nc.gpsimd.tensor_tensor(t3[:], psn_i(i), c0t[:], OP.mult)
                        t4 = tmp.tile([128, T], BF16, tag="rt", bufs=4, name="pt4")
                        nc.gpsimd.tensor_tensor(t4[:], pc_i(i), c1t[:], OP.mult)
                        hi = abv.tile([128, T], BF16, tag="bv", bufs=2, name="hi")
                        nc.vector.tensor_tensor(hi[:], t3[:], t4[:], OP.subtract)
                        yv2 = ypp.tile([128, T], BF16, tag="ypart")
                        nc.vector.tensor_tensor(yv2[:], hi[:], so[CT + i][:],
                                                OP.mult)
                        ys[CT + i] = yv2

                    # ---- lnr + out_proj (k-chains visit yn tiles in their
                    # production order so PE starts during the post window) ----
                    yn = do_ln([t[:] for t in ys], rhs, "lnout", order=PERM[:12])
                    outw_tiles = load_w15(d["w_out"], l, range(2 * CT), 0, CT * 128)

                    def out_cb(m0, psl):
                        for mi, ps in enumerate(psl):
                            mt = m0 + mi
                            xn[mt] = xres.tile([128, T], F32R, tag="x", name="xn")
                            nc.vector.scalar_tensor_tensor(
                                xn[mt][:], ps[:],
                                vt["outbv"][:, co + mt:co + mt + 1],
                                x[mt][:], OP.add, OP.add)

                    proj(2 * CT, CT,
                         lambda ki, mt: outw_tiles[ki][:, mt * 128:(mt + 1) * 128],
                         yn, 3, out_cb, k_order=PERM[:12])

                    # ---- ln2 + MLP (w1/w2 in two hidden-halves; 12 gelu
                    # tiles live at a time) ----
                    ln2r = do_ln([xn[i][:] for i in range(CT)], rhs, "lnout")
                    mparts = {}
                    for q in range(2):
                        w1q = load_whalf(d["w_1"], l, q)
                        gl = [None] * 12

                        def w1_cb(m0, psl, q=q, gl=gl):
                            for mi, ps in enumerate(psl):
                                mt_abs = q * 12 + m0 + mi
                                g = sop.tile([128, T], BF16, tag="gelu", bufs=12)
                                nc.scalar.activation(
                                    g[:], ps[:], AF.Gelu,
                                    bias=vt["b1v"][:, l * 24 + mt_abs:
                                                   l * 24 + mt_abs + 1])
                                gl[m0 + mi] = g

                        proj(CT, 12,
                             lambda ki, mpos, w1q=w1q:
                             w1q[ki][:, mpos * 128:(mpos + 1) * 128],
                             ln2r, 3, w1_cb)
                        w2p = load_w15(d["w_2"], l, range(q * 12, q * 12 + 12),
                                       0, CT * 128)

                        def w2_cb(m0, psl, q=q):
                            for mi, ps in enumerate(psl):
                                mt = m0 + mi
                                if q == 0:
                                    pt = mpp.tile([128, T], BF16, tag="mpart",
                                                  name="mpart")
                                    nc.scalar.activation(pt[:], ps[:], AF.Copy)
                                    mparts[mt] = pt
                                else:
                                    t = tmp.tile([128, T], F32, tag="m2t",
                                                 bufs=2, name="m2t")
                                    nc.vector.tensor_tensor(
                                        t[:], xn[mt][:], mparts[mt][:], OP.add)
                                    nc.vector.scalar_tensor_tensor(
                                        x[mt][:], ps[:],
                                        vt["b2v"][:, co + mt:co + mt + 1],
                                        t[:], OP.add, OP.add)

                        proj(12, CT,
                             lambda ki, mt, w2p=w2p:
                             w2p[ki][:, mt * 128:(mt + 1) * 128],
                             gl, 3, w2_cb)

                # final LN + AllGather of activations
                xf = do_ln([x[i][:] for i in range(CT)], rhs, "lnout")
                for i in range(CT):
                    nc.sync.dma_start(xf_in[i * 128:(i + 1) * 128, :], xf[i][:])
                nc.gpsimd.collective_compute(
                    "AllGather", OP.bypass, replica_groups=ALL8,
                    ins=[xf_in[:]], outs=[xf_all[:]],
                )

            # ---------------- logits phase ----------------
            lst = contextlib.ExitStack()
            with lst:
                pwp = lst.enter_context(tc.tile_pool(name="pwp", bufs=12))
                xfp = lst.enter_context(tc.tile_pool(name="xfp", bufs=48))
                outp_p = lst.enter_context(tc.tile_pool(name="outpp", bufs=8))

                evac_cnt = [0]

                def evac_store(ps, vt_abs, tcol):
                    ot = outp_p.tile([128, T], BF16, tag="ot")
                    if evac_cnt[0] % 2 == 0:
                        nc.scalar.activation(ot[:], ps[:], AF.Copy)
                    else:
                        nc.vector.tensor_copy(ot[:], ps[:])
                    evac_cnt[0] += 1
                    nc.sync.dma_start(
                        outp[vt_abs * 128:(vt_abs + 1) * 128,
                             tcol:tcol + T], ot[:])

                xfb = {}
                for grp_i, (v0, v1) in enumerate(((0, VGRP), (VGRP, VTN))):
                    gw = (v1 - v0) * 128
                    pwtl = []
                    for kt in range(CT):
                        w = pwp.tile([128, VGRP * 128], BF16, tag="pw", name="pwg")
                        nc.sync.dma_start(
                            w[:, :gw], d["pwt"][kt, :, v0 * 128:v0 * 128 + gw])
                        pwtl.append(w)
                    if grp_i == 0:
                        for tb in range(NC):
                            tiles = []
                            for kt in range(CT):
                                xt = xfp.tile([128, T], BF16, tag="xfb")
                                nc.sync.dma_start(
                                    xt[:],
                                    xf_all[tb * D + kt * 128:
                                           tb * D + (kt + 1) * 128, :])
                                tiles.append(xt)
                            xfb[tb] = tiles
                    for tb0 in range(0, NC, 4):
                        for vt in range(v0, v1):
                            vl = vt - v0
                            psl = [ps_mm.tile([128, T], F32, tag="mmps",
                                              name="lgps") for _ in range(4)]
                            for kt in range(CT):
                                for ti in range(4):
                                    tb = tb0 + ti
                                    nc.tensor.matmul(
                                        psl[ti][:],
                                        pwtl[kt][:, vl * 128:(vl + 1) * 128],
                                        xfb[tb][kt][:],
                                        start=(kt == 0), stop=(kt == CT - 1))
                            for ti in range(4):
                                evac_store(psl[ti], vt, (tb0 + ti) * T)
    return d


def _host_prep(inputs):
    f32 = np.float32
    bf = ml_dtypes.bfloat16
    tokens = np.asarray(inputs["tokens"]).astype(np.int64)
    emb = np.asarray(inputs["emb"], dtype=f32)
    theta = np.exp(np.asarray(inputs["theta_log"], dtype=np.float64))
    nu = np.exp(-np.exp(np.asarray(inputs["nu_log"], dtype=np.float64)))
    gamma = np.exp(np.asarray(inputs["gamma_log"], dtype=np.float64))

    def vec_tile(a, per_l):
        a = np.asarray(a, dtype=f32)
        if a.ndim == 1:
            a = a[None, :]
        Ln = a.shape[0]
        out = np.zeros((128, per_l * Ln), f32)
        for l in range(Ln):
            out[:, l * per_l:(l + 1) * per_l] = a[l].reshape(per_l, 128).T
        return out

    def mm_tile(w, ktn, perm=None):
        w = np.asarray(w, dtype=f32)
        Ln, K, M = w.shape
        out = w.reshape(Ln, ktn, 128, M)
        if perm is not None:
            mt = M // 128
            out = out.reshape(Ln, ktn, 128, mt, 128)[:, :, :, perm, :]
            out = out.reshape(Ln, ktn, 128, M)
        return np.ascontiguousarray(out).astype(bf)

    # ---- fold LN gains/biases into adjacent weights (exact, host-side) ----
    # ln(x) @ W + c = ((x-m)*rstd*g + b) @ W + c
    #              = ((x-m)*rstd) @ (diag(g) W) + (b @ W + c)
    ln1_g = np.asarray(inputs["ln1_g"], dtype=np.float64)
    ln1_b = np.asarray(inputs["ln1_b"], dtype=np.float64)
    inw0 = np.array(inputs["inw"], dtype=np.float64)
    inb = np.array(inputs["inb"], dtype=np.float64) + np.einsum(
        "ld,ldm->lm", ln1_b, inw0)
    inw = inw0 * ln1_g[:, :, None]
    # fold gamma into in_proj v columns + bias
    gm = gamma
    inw[:, :, :D] *= gm[:, None, :]
    inw[:, :, D:2 * D] *= gm[:, None, :]
    inb[:, :D] *= gm
    inb[:, D:2 * D] *= gm

    outw = np.array(inputs["outw"], dtype=np.float64)
    outb = np.array(inputs["outb"], dtype=np.float64)
    lnr_g = np.asarray(inputs["lnr_g"], dtype=np.float64)
    lnr_b = np.asarray(inputs["lnr_b"], dtype=np.float64)
    outb = outb + np.einsum("ld,ldm->lm", lnr_b, outw)
    outw = outw * lnr_g[:, :, None]

    w1 = np.array(inputs["w1"], dtype=np.float64)
    b1 = np.array(inputs["b1"], dtype=np.float64)
    ln2_g = np.asarray(inputs["ln2_g"], dtype=np.float64)
    ln2_b = np.asarray(inputs["ln2_b"], dtype=np.float64)
    b1 = b1 + np.einsum("ld,ldm->lm", ln2_b, w1)
    w1 = w1 * ln2_g[:, :, None]

    pw = np.array(inputs["pw"], dtype=np.float64)
    pb = np.array(inputs["pb"], dtype=np.float64)
    lnf_g = np.asarray(inputs["lnf_g"], dtype=np.float64)
    lnf_b = np.asarray(inputs["lnf_b"], dtype=np.float64)
    pb = pb + lnf_b @ pw
    pw = pw * lnf_g[:, None]

    # bmask: scan-boundary zero at col TC
    bmask = np.ones((128, T), f32)
    bmask[:, TC] = 0.0

    # npw table: nu^(tloc+1), [L, D, TC]
    t_loc = np.arange(TC, dtype=np.float64)
    npw = (nu[:, :, None] ** (t_loc[None, None, :] + 1.0)).astype(bf)

    base = {
        "bmask": bmask,
        "nuv": vec_tile(nu.astype(f32), CT),
        "outbv": vec_tile(outb.astype(f32), CT),
        "b2v": vec_tile(inputs["b2"], CT),
        "inbv": vec_tile(inb.astype(f32), 24),
        "b1v": vec_tile(b1.astype(f32), 24),
        "npwt": npw,
        "w_in": mm_tile(inw.astype(f32), CT, perm=PERM),
        "w_out": mm_tile(outw.astype(f32), 2 * CT),
        "w_1": mm_tile(w1.astype(f32), CT),
        "w_2": mm_tile(inputs["w2"], 24),
    }

    tok = tokens.reshape(B, S)
    in_maps = []
    for k in range(NC):
        rows = np.concatenate([tok[0, k * TC:(k + 1) * TC],
                               tok[1, k * TC:(k + 1) * TC]])
        x0t = np.ascontiguousarray(emb[rows].T.astype(f32))
        tg = k * TC + t_loc
        ang = tg[None, None, :] * theta[:, :, None]
        postc = np.cos(ang).astype(bf)
        posts = np.sin(ang).astype(bf)
        cw = np.zeros((L, CT, 128, NCH), f32)
        for j in range(k):
            wj = nu ** (TC * (k - 1 - j))
            cw[:, :, :, j] = wj.reshape(L, CT, 128).astype(f32)
        vs = min(VSH * k, V)
        ve = min(vs + VSH, V)
        pwk = np.zeros((D, VSHP), f32)
        pwk[:, :ve - vs] = pw[:, vs:ve].astype(f32)
        pwt = np.ascontiguousarray(pwk.reshape(CT, 128, VSHP)).astype(bf)
        mm = dict(base)
        mm.update({"x0t": x0t, "postc": postc, "posts": posts,
                   "cw": cw, "pwt": pwt})
        in_maps.append(mm)
    return in_maps, pb.astype(f32)


_CACHE = {}


def _get_nc():
    if "nc" not in _CACHE:
        nc = bacc.Bacc("TRN2", target_bir_lowering=False, debug=False,
                       num_devices=NC)
        _build(nc)
        nc.compile()
        _CACHE["nc"] = nc
    return _CACHE["nc"]


def kernel(**inputs):
    nc = _get_nc()
    in_maps, pb = _host_prep(inputs)
    res = run_bass_kernel_spmd(nc, in_maps, core_ids=list(range(NC)),
                               trace=False)
    out = np.empty((B, S, V), np.float32)
    for k in range(NC):
        vs = min(VSH * k, V)
        ve = min(vs + VSH, V)
        o = np.asarray(res.results[k]["outp"]).astype(np.float32)
        # o: [VSHP, NC*T], col = tb*T + b*TC + tloc
        o4 = o[:ve - vs].reshape(ve - vs, NC, 2, TC)
        for b in range(B):
            out[b, :, vs:ve] = (o4[:, :, b, :].reshape(ve - vs, S).T
                                + pb[vs:ve])
    return out
